# revision 7
# baseline (speedup 1.0000x reference)
"""HalfKP NNUE feature-transformer + MLP head for 8 Trainium2 NeuronCores.

Strategy (data-parallel over batch):
  - Each of the 8 cores gets B/8 = 1024 batch rows of white/black features.
  - Host pre-transposes each feature shard to [K, Bc] so the contraction dim
    (K = 40960) lands on SBUF partitions with fully contiguous DMA reads.
  - Device streams 2 MB chunks (512 feature rows x 1024 batch) and
    accumulates out[4, Bc] = ft_w @ featT in PSUM over 320 k-tiles.
  - The stm blend + clips + l1/l2 layers run on-device on [<=8, 1024] tiles.
  - ft_w is packed on host into per-k-tile lhsT tiles matching the chunk
    interleave: wsb[p, 4*t+m] = ft_w[m, k(t,p)].
"""

import numpy as np

import concourse.bass as bass
import concourse.bacc as bacc_mod
import concourse.mybir as mybir
from concourse.tile import TileContext
from concourse.bass_utils import run_bass_kernel_spmd

N_CORES = 8
B = 8192
K = 40960
M = 4
BC = B // N_CORES        # 1024 batch rows per core
CHUNK = 512              # feature (k) rows per DMA chunk
J = CHUNK // 128         # k-slices per chunk
NCHUNK = K // CHUNK      # 80
NB = BC // 512           # psum halves (matmul free-dim limit is 512 fp32)

MM_DT = mybir.dt.float32  # main matmul dtype (float32 or float32r)
FEAT_DMA = lambda nc: nc.sync  # engine for feature loads

_nc_cache = {}


def _build_nc():
    key = str(MM_DT)
    if key in _nc_cache:
        return _nc_cache[key]
    f32 = mybir.dt.float32
    alu = mybir.AluOpType
    nc = bacc_mod.Bacc(trn_type="TRN2")

    white = nc.dram_tensor("white_t", [NCHUNK, 128, J * BC], MM_DT, kind="ExternalInput")
    black = nc.dram_tensor("black_t", [NCHUNK, 128, J * BC], MM_DT, kind="ExternalInput")
    wsb = nc.dram_tensor("wsb", [128, (K // 128) * M], MM_DT, kind="ExternalInput")
    consts = nc.dram_tensor("consts", [8, 20], f32, kind="ExternalInput")
    stm4 = nc.dram_tensor("stm4", [M, BC], f32, kind="ExternalInput")
    out = nc.dram_tensor("out", [1, BC], f32, kind="ExternalOutput")

    with TileContext(nc) as tc:
        with (
            tc.tile_pool(name="const", bufs=1) as cpool,
            tc.tile_pool(name="feat", bufs=4) as fpool,
            tc.tile_pool(name="psum", bufs=1, space="PSUM") as ppool,
            tc.tile_pool(name="tail", bufs=1) as tpool,
        ):
            w_tile = cpool.tile([128, (K // 128) * M], MM_DT, tag="w")
            nc.sync.dma_start(out=w_tile[:], in_=wsb[:])
            c_tile = cpool.tile([8, 20], f32, tag="c")
            nc.sync.dma_start(out=c_tile[:], in_=consts[:])
            s_tile = cpool.tile([M, BC], f32, tag="s")
            nc.sync.dma_start(out=s_tile[:], in_=stm4[:])

            # accumulators: [4, 1024] fp32 = 2 PSUM banks each
            psums = [ppool.tile([M, BC], f32, tag=f"acc{s}", name=f"acc{s}")
                     for s in range(2)]
            p1 = ppool.tile([8, BC], f32, tag="p1")
            # Warmup matmuls: consume the w_tile/c_tile DMA deps on PE so no
            # later matmul needs two sem waits (fp32 LDW has one wait slot).
            nc.tensor.matmul(psums[0][:, 0:4], w_tile[:, 0:4], w_tile[:, 0:4],
                             start=True, stop=True, skip_group_check=True)
            nc.tensor.matmul(p1[0:8, 0:8], c_tile[0:4, 0:8],
                             c_tile[0:4, 0:8], start=True, stop=True,
                             skip_group_check=True)
            srcs = [white, black]
            for c in range(NCHUNK):
                for s in range(2):
                    ft = fpool.tile([128, J * BC], MM_DT, tag=f"feat{s}", name=f"ft{s}_{c}")
                    FEAT_DMA(nc).dma_start(out=ft[:], in_=srcs[s][c])
                    for j in range(J):
                        t = c * J + j
                        for h in range(NB):
                            nc.tensor.matmul(
                                psums[s][:, h * 512:(h + 1) * 512],
                                w_tile[:, M * t:M * (t + 1)],
                                ft[:, j * BC + h * 512: j * BC + (h + 1) * 512],
                                start=(c == 0 and j == 0),
                                stop=(c == NCHUNK - 1 and j == J - 1),
                            )

            # ---- tail: bias, stm blend, clips, l1, l2 ----
            ftb = c_tile[0:M, 17:18]
            sw = tpool.tile([M, BC], f32, tag="sw")
            sb = tpool.tile([M, BC], f32, tag="sb")
            nc.vector.tensor_scalar_add(out=sw[:], in0=psums[0][:], scalar1=ftb)
            nc.vector.tensor_scalar_add(out=sb[:], in0=psums[1][:], scalar1=ftb)
            diff = tpool.tile([M, BC], f32, tag="diff")
            nc.vector.tensor_sub(out=diff[:], in0=sw[:], in1=sb[:])
            sdiff = tpool.tile([M, BC], f32, tag="sdiff")
            nc.vector.tensor_mul(out=sdiff[:], in0=diff[:], in1=s_tile[:])
            # acc[0:4] = b + stm*(w-b);  acc[4:8] = w - stm*(w-b)
            accA = tpool.tile([M, BC], f32, tag="accA")
            nc.vector.tensor_add(out=accA[:], in0=sb[:], in1=sdiff[:])
            accB = tpool.tile([M, BC], f32, tag="accB")
            nc.vector.tensor_sub(out=accB[:], in0=sw[:], in1=sdiff[:])
            cA = tpool.tile([M, BC], f32, tag="cA")
            nc.vector.tensor_scalar(out=cA[:], in0=accA[:], scalar1=0.0,
                                    scalar2=1.0, op0=alu.max, op1=alu.min)
            cB = tpool.tile([M, BC], f32, tag="cB")
            nc.vector.tensor_scalar(out=cB[:], in0=accB[:], scalar1=0.0,
                                    scalar2=1.0, op0=alu.max, op1=alu.min)
            # l1: out[n, b] = sum_c l1_w[n, c] acc8[c, b], contraction split 4+4
            for h in range(NB):
                sl = slice(h * 512, (h + 1) * 512)
                nc.tensor.matmul(p1[:, sl], c_tile[0:4, 0:8], cA[:, sl],
                                 start=True, stop=False)
                nc.tensor.matmul(p1[:, sl], c_tile[0:4, 8:16], cB[:, sl],
                                 start=False, stop=True)
            l1x = tpool.tile([8, BC], f32, tag="l1x")
            nc.vector.tensor_scalar_add(out=l1x[:], in0=p1[:],
                                        scalar1=c_tile[0:8, 18:19])
            l1c = tpool.tile([8, BC], f32, tag="l1c")
            nc.vector.tensor_scalar(out=l1c[:], in0=l1x[:], scalar1=0.0,
                                    scalar2=1.0, op0=alu.max, op1=alu.min)
            p2 = ppool.tile([1, BC], f32, tag="p2")
            for h in range(NB):
                sl = slice(h * 512, (h + 1) * 512)
                nc.tensor.matmul(p2[:, sl], c_tile[0:8, 16:17], l1c[:, sl],
                                 start=True, stop=True)
            ot = tpool.tile([1, BC], f32, tag="ot")
            nc.vector.tensor_scalar_add(out=ot[:], in0=p2[:],
                                        scalar1=c_tile[0:1, 19:20])
            nc.sync.dma_start(out=out[:], in_=ot[:])

    nc.finalize()
    _nc_cache[key] = nc
    return nc


def _prep_inputs(white_features, black_features, stm, ft_w, ft_b, l1_w, l1_b,
                 l2_w, l2_b):
    white_features = np.asarray(white_features, np.float32)
    black_features = np.asarray(black_features, np.float32)
    stm = np.asarray(stm, np.float32)
    ft_w = np.asarray(ft_w, np.float32)
    ft_b = np.asarray(ft_b, np.float32)
    l1_w = np.asarray(l1_w, np.float32)
    l1_b = np.asarray(l1_b, np.float32)
    l2_w = np.asarray(l2_w, np.float32)
    l2_b = np.asarray(l2_b, np.float32)

    # wsb[p, 4t+m] = ft_w[m, k(t,p)] with k(t,p) = c*CHUNK + J*p + j, t = c*J+j
    ftwT = np.ascontiguousarray(ft_w.T)  # [K, 4]
    wsb = (ftwT.reshape(NCHUNK, 128, J, M)
           .transpose(1, 0, 2, 3).reshape(128, (K // 128) * M).copy())

    consts = np.zeros((8, 20), np.float32)
    consts[0:4, 0:8] = l1_w[:, 0:4].T
    consts[0:4, 8:16] = l1_w[:, 4:8].T
    consts[0:8, 16] = l2_w[0, :]
    consts[0:4, 17] = ft_b
    consts[0:8, 18] = l1_b
    consts[0, 19] = l2_b[0]

    in_maps = []
    for c in range(N_CORES):
        sl = slice(c * BC, (c + 1) * BC)
        wt = np.ascontiguousarray(white_features[sl].T).reshape(NCHUNK, 128, J * BC)
        bt = np.ascontiguousarray(black_features[sl].T).reshape(NCHUNK, 128, J * BC)
        stm4 = np.ascontiguousarray(
            np.broadcast_to(stm[sl][None, :], (M, BC)))
        in_maps.append({
            "white_t": wt, "black_t": bt, "wsb": wsb,
            "consts": consts, "stm4": stm4,
        })
    return in_maps


def _run(in_maps, trace=False, **kw):
    nc = _build_nc()
    res = run_bass_kernel_spmd(nc, in_maps, core_ids=list(range(N_CORES)),
                               trace=trace, **kw)
    out = np.concatenate(
        [r["out"].reshape(BC, 1) for r in res.results], axis=0)
    return out, res


def kernel(**inputs):
    in_maps = _prep_inputs(**inputs)
    out, _ = _run(in_maps, trace=False)
    return out
